# revision 6
# baseline (speedup 1.0000x reference)
"""AdaptiveAngleConv Trainium2 kernel — 8-core data-parallel Bass/Tile.

Per-sample dynamic 3x3 conv (256->256ch, 80x80) with attention-synthesized
weights. Batch 16 is sharded 2 samples/core across 8 NeuronCores.

All compute (pooling, attention, weight synthesis, conv) runs on device.
Host-side work is layout-only: sharding x, pre-transposing the small weight
tensors, and replicating them per core.
"""

import numpy as np

import concourse.bass as bass
import concourse.mybir as mybir
import concourse.tile as tile
from concourse import bacc
from concourse.bass_utils import run_bass_kernel_spmd

# ---------------------------------------------------------------- constants
P = 128
BS, CIN, COUT, H, W = 16, 256, 256, 80, 80
HID, K, TEMP = 16, 5, 30.0
NCORES = 8
BSL = BS // NCORES            # samples per core
CB = CIN // P                 # cin partition blocks
OC = COUT // P                # cout partition blocks
R_TILE = 5                    # output rows per psum tile (5*80=400 <= 512)

# clockwise ring order of the 8 non-center taps of a 3x3 kernel (flat idx)
RING = [0, 1, 2, 5, 8, 7, 6, 3]
SHIFTS = [0, 1, 2, 3, 4]      # ring shifts for angles 0/45/90/135/180

F32 = mybir.dt.float32
F32R = mybir.dt.float32r

AF = mybir.ActivationFunctionType
ALU = mybir.AluOpType
AX = mybir.AxisListType


# ---------------------------------------------------------------- builder
def _emit(tc, aps, dt_mm, h, w, bsl):
    nc = tc.nc
    hp, wp = h + 2, w + 2
    ntiles = h // R_TILE
    assert h % R_TILE == 0
    ngroups = max(1, ntiles // 4)
    assert ntiles % min(ntiles, 4) == 0
    gsize = ntiles // ngroups

    (x_d, wring_d, attcat_d, wnetT_d, wcinT_d, woutT_d, bsumT_d,
     ident_d, ones_d, zeros_d, out_d) = aps

    DT = dt_mm  # dtype for matmul operand tiles (F32R or F32)

    def mm(out, lhsT, rhs, start, stop):
        nc.tensor.matmul(out, lhsT, rhs, start=start, stop=stop)

    import contextlib
    with contextlib.ExitStack() as ctx:
        persist = ctx.enter_context(tc.tile_pool(name="persist", bufs=1))
        diagp = ctx.enter_context(tc.tile_pool(name="diagp", bufs=6))
        stagep = ctx.enter_context(tc.tile_pool(name="stagep", bufs=4))
        cps = ctx.enter_context(tc.tile_pool(name="cps", bufs=4, space="PSUM"))
        sps = ctx.enter_context(tc.tile_pool(name="sps", bufs=2, space="PSUM"))
        atp = ctx.enter_context(tc.tile_pool(name="atp", bufs=2, space="PSUM"))

        # ------------------------------------------------ persistent tiles
        wring = [persist.tile([P, 17, COUT], DT, name=f"wring{cb}")
                 for cb in range(CB)]
        wnetT = [persist.tile([P, HID], F32, name=f"wnetT{cb}")
                 for cb in range(CB)]
        attcat = persist.tile([HID, K + 9], F32, name="attcat")
        wcinT = persist.tile([HID, CIN], F32, name="wcinT")
        woutT = persist.tile([HID, COUT], F32, name="woutT")
        bsumT = [persist.tile([P, K], F32, name=f"bsumT{o}") for o in range(OC)]
        ident = persist.tile([P, P], DT, name="ident")
        ones = persist.tile([1, P], F32, name="ones")
        zeros = persist.tile([P, max(h, w) + 2], DT, name="zeros")
        xp = [[persist.tile([P, hp, wp], DT, name=f"xp{b}_{cb}")
               for cb in range(CB)] for b in range(bsl)]
        agg = [[persist.tile([P, 9, COUT], DT, name=f"agg{b}_{cb}")
                for cb in range(CB)] for b in range(bsl)]
        pooled = [persist.tile([P, bsl], F32, name=f"pooled{cb}")
                  for cb in range(CB)]
        cin_att = [persist.tile([P, bsl], F32, name=f"cina{cb}")
                   for cb in range(CB)]
        out_att = [persist.tile([P, bsl], F32, name=f"outa{o}")
                   for o in range(OC)]
        aggb = [persist.tile([P, bsl], F32, name=f"aggb{o}") for o in range(OC)]
        natt = [persist.tile([P, K + 9], F32, name=f"natt{b}")
                for b in range(bsl)]
        hsb = [persist.tile([HID, 1], F32, name=f"h{b}") for b in range(bsl)]
        rowsb = [persist.tile([1, K + 9], F32, name=f"row{b}")
                 for b in range(bsl)]
        mx = persist.tile([1, 2], F32, name="mx")

        # ------------------------------------------------ const DMAs
        for cb in range(CB):
            nc.sync.dma_start(wring[cb][:], wring_d[cb])
            nc.sync.dma_start(wnetT[cb][:], wnetT_d[cb])
        nc.sync.dma_start(attcat[:], attcat_d[:])
        nc.sync.dma_start(wcinT[:], wcinT_d[:])
        nc.sync.dma_start(woutT[:], woutT_d[:])
        for o in range(OC):
            nc.sync.dma_start(bsumT[o][:], bsumT_d[o])
        nc.sync.dma_start(ident[:], ident_d[:])
        nc.sync.dma_start(ones[:], ones_d[:])
        nc.sync.dma_start(zeros[:], zeros_d[:])

        # ------------------------------------------------ x load + pad
        half = h // 2
        for b in range(bsl):
            for cb in range(CB):
                t = xp[b][cb]
                nc.sync.dma_start(t[:, 0, :], zeros[:, 0:wp])
                nc.sync.dma_start(t[:, hp - 1, :], zeros[:, 0:wp])
                nc.sync.dma_start(t[:, 1:hp - 1, 0:1], zeros[:, 0:h])
                nc.sync.dma_start(t[:, 1:hp - 1, wp - 1:wp], zeros[:, 0:h])
                nc.sync.dma_start(t[:, 1:1 + half, 1:w + 1],
                                  x_d[b, cb * P:(cb + 1) * P, 0:half, :])
                nc.sync.dma_start(t[:, 1 + half:h + 1, 1:w + 1],
                                  x_d[b, cb * P:(cb + 1) * P, half:h, :])

        # ------------------------------------------------ pooling
        for b in range(bsl):
            for cb in range(CB):
                nc.vector.tensor_reduce(pooled[cb][:, b:b + 1],
                                        xp[b][cb][:].bitcast(F32),
                                        axis=AX.XY, op=ALU.add)

        # ------------------------------------------------ attention
        for b in range(bsl):
            ph = atp.tile([HID, 1], F32, tag="a", name=f"ph{b}")
            for cb in range(CB):
                nc.tensor.matmul(ph[:], wnetT[cb][:], pooled[cb][:, b:b + 1],
                                 start=(cb == 0), stop=(cb == CB - 1))
            nc.scalar.activation(hsb[b][:], ph[:], AF.Relu, scale=1.0 / (h * w))

            prow = atp.tile([1, K + 9], F32, tag="a", name=f"prow{b}")
            nc.tensor.matmul(prow[:], hsb[b][:], attcat[:], start=True, stop=True)
            # softmax over cols 0:K (with temperature)
            nc.vector.reduce_max(mx[:, 0:1], prow[:, 0:K], axis=AX.X)
            nc.vector.tensor_scalar(mx[:, 1:2], mx[:, 0:1], -1.0 / TEMP, None,
                                    op0=ALU.mult)
            nc.scalar.activation(rowsb[b][:, 0:K], prow[:, 0:K], AF.Exp,
                                 bias=mx[:, 1:2], scale=1.0 / TEMP)
            nc.vector.reduce_sum(mx[:, 0:1], rowsb[b][:, 0:K], axis=AX.X)
            nc.vector.reciprocal(mx[:, 1:2], mx[:, 0:1])
            nc.vector.tensor_scalar(rowsb[b][:, 0:K], rowsb[b][:, 0:K],
                                    mx[:, 1:2], None, op0=ALU.mult)
            # sigmoid for k2 cols
            nc.scalar.activation(rowsb[b][:, K:K + 9], prow[:, K:K + 9],
                                 AF.Sigmoid)
            # broadcast row across 128 partitions
            pb = atp.tile([P, K + 9], F32, tag="a", name=f"pb{b}")
            nc.tensor.matmul(pb[:], ones[:], rowsb[b][:], start=True, stop=True)
            nc.vector.tensor_copy(natt[b][:], pb[:])

            for cb in range(CB):
                pc = atp.tile([P, 1], F32, tag="a", name=f"pc{b}_{cb}")
                nc.tensor.matmul(pc[:], wcinT[:, cb * P:(cb + 1) * P],
                                 hsb[b][:], start=True, stop=True)
                nc.scalar.activation(cin_att[cb][:, b:b + 1], pc[:], AF.Sigmoid)
            for o in range(OC):
                po = atp.tile([P, 1], F32, tag="a", name=f"po{b}_{o}")
                nc.tensor.matmul(po[:], woutT[:, o * P:(o + 1) * P],
                                 hsb[b][:], start=True, stop=True)
                nc.scalar.activation(out_att[o][:, b:b + 1], po[:], AF.Sigmoid)
                tmp5 = diagp.tile([P, K], F32, tag="tmp5", name=f"tmp5_{b}_{o}")
                nc.vector.tensor_tensor(tmp5[:], bsumT[o][:], natt[b][:, 0:K],
                                        op=ALU.mult)
                nc.vector.reduce_sum(aggb[o][:, b:b + 1], tmp5[:], axis=AX.X)

        # ------------------------------------------------ weight synthesis
        for b in range(bsl):
            for cb in range(CB):
                diags = []
                for k in range(K):
                    dg = diagp.tile([P, P], DT, tag="diag", name=f"dg{b}_{cb}_{k}")
                    nc.vector.tensor_scalar(dg[:], ident[:].bitcast(F32),
                                            natt[b][:, k:k + 1],
                                            cin_att[cb][:, b:b + 1],
                                            op0=ALU.mult, op1=ALU.mult)
                    diags.append(dg)
                dgc = diagp.tile([P, P], DT, tag="diag", name=f"dgc{b}_{cb}")
                nc.vector.tensor_scalar(dgc[:], ident[:].bitcast(F32),
                                        cin_att[cb][:, b:b + 1], None,
                                        op0=ALU.mult)
                for j in range(4):
                    ps = sps.tile([P, 512], F32, tag="s", name=f"ps{b}_{cb}_{j}")
                    for k, s in enumerate(SHIFTS):
                        rhs = wring[cb][:, 8 - s + 2 * j: 10 - s + 2 * j, :]
                        mm(ps[:], diags[k][:], rhs, k == 0, k == K - 1)
                    for hf in range(2):
                        q = RING[2 * j + hf]
                        nc.vector.tensor_scalar(
                            agg[b][cb][:, 2 * j + hf, :],
                            ps[:, hf * 256:(hf + 1) * 256],
                            natt[b][:, K + q:K + q + 1], None, op0=ALU.mult)
                psc = sps.tile([P, 512], F32, tag="s", name=f"psc{b}_{cb}")
                mm(psc[:, 0:256], dgc[:], wring[cb][:, 16, :], True, True)
                nc.vector.tensor_scalar(agg[b][cb][:, 8, :], psc[:, 0:256],
                                        natt[b][:, K + 4:K + 5], None,
                                        op0=ALU.mult)

        # ------------------------------------------------ conv + epilogue
        for b in range(bsl):
            for o in range(OC):
                for g in range(ngroups):
                    pts = [cps.tile([P, R_TILE, w], F32, tag="c",
                                     name=f"pt{b}_{o}_{g}_{i}")
                           for i in range(gsize)]
                    for islot in range(9):
                        q = RING[islot] if islot < 8 else 4
                        dy, dx = q // 3, q % 3
                        for cb in range(CB):
                            lhsT = agg[b][cb][:, islot, o * P:(o + 1) * P]
                            for ti in range(gsize):
                                t = g * gsize + ti
                                rhs = xp[b][cb][:, t * R_TILE + dy:
                                                t * R_TILE + dy + R_TILE,
                                                dx:dx + w]
                                mm(pts[ti][:], lhsT, rhs,
                                   islot == 0 and cb == 0,
                                   islot == 8 and cb == CB - 1)
                    for ti in range(gsize):
                        t = g * gsize + ti
                        st = stagep.tile([P, R_TILE, w], F32, tag="stage", name=f"st{b}_{o}_{t}")
                        nc.vector.tensor_scalar(st[:], pts[ti][:],
                                                out_att[o][:, b:b + 1],
                                                aggb[o][:, b:b + 1],
                                                op0=ALU.mult, op1=ALU.add)
                        nc.sync.dma_start(
                            out_d[b, o * P:(o + 1) * P,
                                  t * R_TILE:(t + 1) * R_TILE, :], st[:])


def build_graph(dt_mm=F32R, h=H, w=W, bsl=BSL):
    nc = bacc.Bacc("TRN2", target_bir_lowering=False, debug=False,
                   num_devices=NCORES)
    aps = (
        nc.dram_tensor("x", [bsl, CIN, h, w], dt_mm, kind="ExternalInput").ap(),
        nc.dram_tensor("w_ring2", [CB, P, 17, COUT], dt_mm,
                       kind="ExternalInput").ap(),
        nc.dram_tensor("att_cat", [HID, K + 9], F32, kind="ExternalInput").ap(),
        nc.dram_tensor("w_netT", [CB, P, HID], F32, kind="ExternalInput").ap(),
        nc.dram_tensor("w_cinT", [HID, CIN], F32, kind="ExternalInput").ap(),
        nc.dram_tensor("w_outT", [HID, COUT], F32, kind="ExternalInput").ap(),
        nc.dram_tensor("b_sumT", [OC, P, K], F32, kind="ExternalInput").ap(),
        nc.dram_tensor("ident", [P, P], dt_mm, kind="ExternalInput").ap(),
        nc.dram_tensor("ones", [1, P], F32, kind="ExternalInput").ap(),
        nc.dram_tensor("zeros", [P, max(w, h) + 2], dt_mm,
                       kind="ExternalInput").ap(),
        nc.dram_tensor("out", [bsl, COUT, h, w], F32, kind="ExternalOutput").ap(),
    )
    with tile.TileContext(nc) as tc:
        _emit(tc, aps, dt_mm, h, w, bsl)
    nc.compile()
    return nc


# ---------------------------------------------------------------- host prep
def round_f32r(a):
    """Round float32 array to fp32r (8-bit exp, 11-bit mantissa, RNE)."""
    u = np.ascontiguousarray(a, dtype=np.float32).view(np.uint32)
    rem = u & np.uint32(0xFFF)
    half = np.uint32(0x800)
    lsb = (u >> np.uint32(12)) & np.uint32(1)
    add = ((rem > half) | ((rem == half) & (lsb == 1))).astype(np.uint32)
    out = ((u & np.uint32(0xFFFFF000)) + (add << np.uint32(12))).astype(np.uint32)
    return out.view(np.float32)


def prep_consts(w_base, b_base, b_extra, w_net, w_nfc, w_cin, w_k2, w_out,
                use_f32r=True):
    f = np.float32
    wflat = w_base.reshape(COUT, CIN, 9).astype(f)
    wT = np.ascontiguousarray(wflat.transpose(1, 2, 0))      # [cin, 9, cout]
    ringidx = RING + RING + [4]                               # 17 slots
    wring = np.ascontiguousarray(wT[:, ringidx, :])           # [cin, 17, cout]
    return {
        "w_ring2": (round_f32r(wring) if use_f32r else wring)
            .reshape(CB, P, 17, COUT),
        "att_cat": np.ascontiguousarray(
            np.concatenate([w_nfc.T, w_k2.T], axis=1).astype(f)),
        "w_netT": np.ascontiguousarray(w_net.T.astype(f)).reshape(CB, P, HID),
        "w_cinT": np.ascontiguousarray(w_cin.T.astype(f)),
        "w_outT": np.ascontiguousarray(w_out.T.astype(f)),
        "b_sumT": np.ascontiguousarray(
            np.concatenate([b_base[None], b_extra], axis=0).T.astype(f)
        ).reshape(OC, P, K),
        "ident": np.eye(P, dtype=f),
        "ones": np.ones((1, P), dtype=f),
        "zeros": np.zeros((P, max(H, W) + 2), dtype=f),
    }


_CACHE = {}


def kernel(**inputs):
    if "nc" not in _CACHE:
        _CACHE["nc"] = build_graph()
        _CACHE.setdefault("f32r", True)
    nc = _CACHE["nc"]
    use_f32r = _CACHE.get("f32r", True)
    x = round_f32r(inputs["x"]) if use_f32r else \
        np.ascontiguousarray(inputs["x"], dtype=np.float32)
    consts = prep_consts(
        inputs["w_base"], inputs["b_base"], inputs["b_extra"],
        inputs["w_net"], inputs["w_nfc"], inputs["w_cin"],
        inputs["w_k2"], inputs["w_out"], use_f32r=use_f32r)
    in_maps = [dict(consts, x=x[i * BSL:(i + 1) * BSL]) for i in range(NCORES)]
    res = run_bass_kernel_spmd(nc, in_maps, list(range(NCORES)),
                               **_CACHE.get("run_kwargs", {}))
    _CACHE["last_result"] = res
    out = np.concatenate([res.results[i]["out"] for i in range(NCORES)], axis=0)
    return out.astype(np.float32)
